# revision 1
# baseline (speedup 1.0000x reference)
"""Trainium2 Bass kernel for AdaBiDiff GNN message passing.

Data parallel over batch B=8, one batch element per core.  Per core:
  xt (12,1536) -> softmax over t -> p, logp (t-major)
  kl[i,j] = rowterm[i] - sum_t p[i,t] logp[j,t];  A = (kl < 0.5)
  u_fwd = (A @ xt.T) / rowsum(A);  u_bwd = (A.T @ xt.T) / colsum(A)
  x_flat[n, t*64+h] = relu(xt[t,n] W1[h] + (0.9 u_fwd + 2.1 u_bwd)[n,t] W2[h])
  two MLP blocks (BN folded into weights on host) -> out (12,1536) per core.

Implementation notes:
  - all weights are baked into the NEFF as inline Const tensors (loaded to
    HBM once at model-load time); the only per-call transfers are x in and
    out back.  The jitted SPMD executable and device-resident zero output
    buffers are cached across kernel() calls; weight content changes are
    detected by fingerprint (id fast path) and trigger a rebuild.
  - one packed (128, CW) f32 weight blob -> a single in-kernel DMA; each
    weight is an SBUF column-slice view of the blob.
  - the KL adjacency compare is computed SCALED by s[i] = Sum_t exp(x[t,i])
    (> 0, so A = (Ghat > 0) is unchanged):
      s*Ghat[i,j] = Sum_t ex[t,i]x[t,j] + s[i]*(-L[j]) + cmb[i]*1,
    with L = ln(s), cmb = (0.5+L)*s - W, W = Sum_t ex*x.  The K=34 operand
    stacks are therefore raw rows: phat = [ex;0..;s@32;cmb@33] and
    xs = [xt;0..;-L@32;1@33], duplicated at partitions 64..97 so the two
    orientations run row-packed (tile_position (0,0) vs (64,0)) on the PE.
    s/cmb land at rows 32-33/96-97 via partition-strided 2-row DMAs; the
    softmax itself (p, logp) is never materialized.
  - A-orientation compare on DVE (is_gt -> 0/1); AT-orientation on ScalarE
    (Sign -> -1/0/1), with the sign-affine correction folded into the
    u_fwd scaling: yA=(yA'+Sx)/2, rs=(rs'+N)/2 -> uf=(yA'+Sx)/(rs'+N).
  - ones column in the transposed-x stationary produces row/col sums free.
  - both product accumulators share PSUM banks (partitions 0-32 and 64-96
    of one 3-bank tile), letting the Ghat tiles double-buffer.
  - stages B-F run as a per-512-column-chunk pipeline: chunk c+1's
    PE-heavy adjacency/products overlap chunk c's DVE/Act-heavy
    normalization and MLP tail.  uf/ub land in a separate xd stack
    ([xt;0;uf;0;ub], rows 0-75) so the writes never collide with stage-B
    reads of xs; x_flat is one K=76 matmul per (k,c) against the
    [e1t;0;e2a;0;e2b] stack in the blob (zero gap rows satisfy the
    32-alignment rule and cost no PE time - matmul time scales with N,
    not K).
  - matmul dtype float32r (1 col/cycle); A/AT tiles and xtT in bf16.
"""

import numpy as np

import concourse.bass as bass
import concourse.bacc as bacc
import concourse.tile as tile
import concourse.mybir as mybir

F32 = mybir.dt.float32
F32R = mybir.dt.float32r
BF16 = mybir.dt.bfloat16
AF = mybir.ActivationFunctionType
ALU = mybir.AluOpType

B, T, N, H, TH, HID2, TOUT = 8, 12, 1536, 64, 768, 128, 12
NT = N // 128
NC = N // 512
AUG = 32

# ---- packed weight blob column layout ----
O_EW1 = 0              # 6 x 128 cols, rows 0-127
O_EPROJ = 768          # 6 x 64 cols, rows 0-127
O_EW2 = 1152           # 128 cols, rows 0-127
O_EW3 = 1280           # 64 cols, rows 0-127
O_DW1 = 1344           # 128 cols, rows 0-63
O_DW2 = 1472           # 128 cols, rows 0-127
O_DW3 = 1600           # 12 cols, rows 0-127
O_DPROJ = 1612         # 12 cols, rows 0-63
O_ES = 1624            # 768 cols, rows 0-75 ([e1t;0;e2a;0;e2b] stack)
O_I12 = 2392           # 12 cols, rows 0-11 (identity)
O_EB1 = 2404           # bias columns (f32 bits)
O_EB2 = 2405
O_EBE = 2406
O_DB1 = 2407
O_DB2 = 2408
O_DBD = 2409
CW = 2410

_cache = {}


def _build_nc(wblob):
    nc = bacc.Bacc("TRN2", target_bir_lowering=False, debug=False)
    d = {}
    d["x"] = nc.declare_dram_parameter("x", [T, N], F32R, isOutput=False)
    d["out"] = nc.declare_dram_parameter("out", [T, N], F32, isOutput=True)
    d["wb"] = nc.inline_tensor(wblob, name="wb")
    # xs rows 12..63 fill: zeros with a ones row where row 33 lands
    zc1 = np.zeros((52, N), np.float32)
    zc1[33 - 12, :] = 1.0
    d["zc1"] = nc.inline_tensor(zc1, name="zc1")
    # xs rows 76..97 fill: zeros with a ones row where row 97 lands
    zc2 = np.zeros((22, N), np.float32)
    zc2[97 - 76, :] = 1.0
    d["zc2"] = nc.inline_tensor(zc2, name="zc2")

    with tile.TileContext(nc) as tc:
        _kernel_body(tc, d)
    nc.compile()
    return nc


def _kernel_body(tc, d):
    nc = tc.nc
    CS = [slice(c * 512, (c + 1) * 512) for c in range(NC)]

    with tc.tile_pool(name="w", bufs=1) as w, tc.tile_pool(name="sb", bufs=1) as sb:

        def stile(name, shape, dt=F32R):
            return sb.tile(list(shape), dt, name=name, tag=name)

        # ---- per-call input + weight blob (x first: it gates the whole chain) ----
        # xs is the Ghat j-side operand stack (read-only after stage A):
        #   rows 0-11 xt | 12-31 zero | 32 -L | 33 one | 34-63 zero |
        #   64-75 xt-dup | 76-95 zero | 96 -L-dup | 97 one-dup
        xs = stile("xs", (98, N))
        nc.sync.dma_start(out=xs[0:T, :], in_=d["x"].ap())
        wb = w.tile([128, CW], F32R, name="wb", tag="wb")
        nc.sync.dma_start(out=wb[:].bitcast(F32), in_=d["wb"].ap())
        xt = xs[0:T, :]
        i12 = wb[0:T, O_I12:O_I12 + T]
        eb1 = wb[:, O_EB1:O_EB1 + 1].bitcast(F32)
        eb2 = wb[:, O_EB2:O_EB2 + 1].bitcast(F32)
        ebe = wb[0:H, O_EBE:O_EBE + 1].bitcast(F32)
        db1 = wb[:, O_DB1:O_DB1 + 1].bitcast(F32)
        db2 = wb[:, O_DB2:O_DB2 + 1].bitcast(F32)
        dbd = wb[0:TOUT, O_DBD:O_DBD + 1].bitcast(F32)

        ones12 = w.tile([T, 1], F32R, name="ones12", tag="ones12")
        nc.vector.memset(ones12[:].bitcast(F32), 1.0)
        ones1 = w.tile([1, T], F32R, name="ones1", tag="ones1")
        nc.vector.memset(ones1[:].bitcast(F32), 1.0)
        # stage-C staging bias column: rows 0-11 = Sx (filled later), row 32 = N
        bSx = w.tile([33, 1], F32, name="bSx", tag="bSx")
        nc.vector.memset(bSx[:], 0.0)
        nc.vector.memset(bSx[32:33, :], float(N))
        ph5 = w.tile([1, 1], F32, name="ph5", tag="ph5")
        nc.vector.memset(ph5[:], 0.5)
        # prewarm the exp activation table under the input DMAs
        warm = w.tile([1, 1], F32, name="warm", tag="warm")
        nc.vector.memset(warm[:], 1.0)
        nc.scalar.activation(warm[:], warm[:], AF.Exp)

        # =========== Stage A ===========
        # Ghat is computed SCALED by s[i] = Sum_t ex[t,i] > 0 (compare vs 0 is
        # unchanged):  s*Ghat[i,j] = Sum_t ex[t,i]*x[t,j]
        #                          + s[i]*(-L[j]) + ((0.5+L[i])*s[i] - W[i])*1
        # with L = ln(s), W = Sum_t ex*x.  So the K=34 operand stacks are raw
        # rows: phat = [ex; 0..; s@32; combo@33], xs = [xt; 0..; -L@32; 1@33],
        # both duplicated at partitions 64..97 for PE row-packing.
        phat = stile("phat", (98, N))
        xtT = stile("xtT", (128, NT, AUG + 1), BF16)

        nc.gpsimd.memset(phat[0:33, :].bitcast(F32), 0.0)
        nc.gpsimd.memset(phat[64:97, :].bitcast(F32), 0.0)
        nc.gpsimd.memset(xtT[:], 0.0)
        # xs zero/one fills (rows 12-63, 76-97) and the xt dup at 64-75
        nc.gpsimd.dma_start(out=xs[T:64, :].bitcast(F32), in_=d["zc1"].ap())
        nc.gpsimd.dma_start(out=xs[76:98, :].bitcast(F32), in_=d["zc2"].ap())
        nc.sync.dma_start(out=xs[64:76, :], in_=d["x"].ap())
        # xd: stage-D moving stack [xt; 0; uf; 0; ub] — separate from xs so
        # per-chunk uf/ub writes don't collide with stage-B reads of xs
        xd = stile("xd", (76, N))
        nc.gpsimd.memset(xd[:].bitcast(F32), 0.0)
        nc.sync.dma_start(out=xd[0:T, :], in_=d["x"].ap())

        with tc.tile_pool(name="pa1", bufs=1, space="PSUM") as pa1, \
             tc.tile_pool(name="pat", bufs=1, space="PSUM") as pat:
            nc.scalar.activation(phat[0:T, :], xt, AF.Exp)
            # hidden Ln-table load while the psA matmuls run
            nc.scalar.activation(warm[:], warm[:], AF.Ln)
            wx = stile("wx", (T, N))
            nc.vector.tensor_tensor(wx[:], phat[0:T, :], xt, ALU.mult)
            # duplicate ex rows for the row-packed orientation
            nc.sync.dma_start(out=phat[64:76, :], in_=phat[0:T, :])

            psA = pa1.tile([1, NC, 512], F32, name="psA", tag="psA")
            psW = pa1.tile([1, NC, 512], F32, name="psW", tag="psW")
            for c in range(NC):
                nc.tensor.matmul(psA[:, c, :], ones12[:], phat[0:T, CS[c]],
                                 start=True, stop=True)
            for c in range(NC):
                nc.tensor.matmul(psW[:, c, :], ones12[:], wx[:, CS[c]],
                                 start=True, stop=True)

            # augP stages [s @ row 0; combo @ row 32]; one strided 2-row DMA per
            # row-packing copy then lands them at phat rows 32-33 / 96-97.
            L = stile("L", (1, N), F32)
            cmb = stile("cmb", (1, N), F32)
            augP = stile("augP", (33, N), F32)
            nc.scalar.activation(L[:], psA[:], AF.Ln)
            nc.scalar.activation(augP[0:1, :], psA[:], AF.Identity)
            # combo = (0.5 + L)*s - W  (chain first: it gates the aug-row DMAs;
            # L2 = L+0.5 also feeds the -L rows so they schedule after it)
            nc.vector.tensor_scalar(cmb[:], L[:], 0.5, None, ALU.add)
            nc.vector.tensor_tensor(augP[32:33, :], cmb[:], augP[0:1, :], ALU.mult)
            nc.vector.tensor_tensor(augP[32:33, :], augP[32:33, :], psW[:], ALU.subtract)
            nc.sync.dma_start(out=phat[32:34, :].bitcast(F32), in_=augP[0:33:32, :])
            nc.scalar.dma_start(out=phat[96:98, :].bitcast(F32), in_=augP[0:33:32, :])
            # xs aug row 32: -L (both row-packing copies; on Act to keep the
            # DVE free for the combo chain that gates the aug-row DMAs)
            nc.scalar.activation(xs[AUG:AUG + 1, :], L[:], AF.Identity, scale=-1.0)
            nc.scalar.activation(xs[96:97, :], L[:], AF.Identity, scale=-1.0)

            # transposed x with ones column (bf16): xtT[p, j, t] = xt[t, 128j+p]
            psT = pat.tile([128, NT, T], F32, name="psT", tag="psT")
            for j in range(NT):
                nc.tensor.matmul(psT[:, j, :], xt[:, j * 128:(j + 1) * 128], i12,
                                 start=True, stop=True)
            nc.vector.tensor_copy(xtT[:, :, 0:T], psT[:])
            nc.vector.memset(xtT[:, :, AUG:AUG + 1], 1.0)

        # =========== Stages B-F: per-chunk pipeline ===========
        # Each 512-column chunk runs adjacency+products (B), normalization
        # (C), x_flat (D) and the MLPs (E/F) independently, so chunk c+1's
        # PE-heavy stage B overlaps chunk c's DVE/Act-heavy tail.  uf/ub go
        # into the separate xd stack (xs stays read-only after stage A).
        nc.vector.tensor_reduce(bSx[0:T, :], xt, mybir.AxisListType.X, ALU.add)
        zT = stile("zT", (128, 6, N))
        h1 = stile("h1", (HID2, N))
        h2 = stile("h2", (HID2, N))
        xe = stile("xe", (H, N))
        g1 = stile("g1", (HID2, N))
        g2 = stile("g2", (HID2, N))
        od = stile("od", (TOUT, N), F32)
        vf = stile("vf", (33, N), F32)
        vb = stile("vb", (33, N), F32)
        rr = stile("rr", (1, N), F32R)
        cc = stile("cc", (1, N), F32R)

        with tc.tile_pool(name="pp", bufs=1, space="PSUM") as pp, \
             tc.tile_pool(name="ab", bufs=5) as ab, \
             tc.tile_pool(name="pgg", bufs=2, space="PSUM") as pgg, \
             tc.tile_pool(name="pgt", bufs=3, space="PSUM") as pgt, \
             tc.tile_pool(name="pf", bufs=2, space="PSUM") as pf:
            for c in range(NC):
                # ---- B(c): Ghat both orientations, compares, products ----
                prodc = pp.tile([128, 512], F32, name="prodc", tag="prod")
                for i in range(NT):
                    isl = slice(i * 128, (i + 1) * 128)
                    Ai = ab.tile([128, 512], BF16, name="Ai", tag="Ai")
                    ATi = ab.tile([128, 512], BF16, name="ATi", tag="ATi")
                    psG = pgg.tile([128, 512], F32, name="psG", tag="psG")
                    nc.tensor.matmul(psG[:], phat[0:34, isl], xs[0:34, CS[c]],
                                     start=True, stop=True, tile_position=(0, 0))
                    nc.vector.tensor_scalar(Ai[:], psG[:], 0.0, None, ALU.is_gt)
                    psGT = pgt.tile([128, 512], F32, name="psGT", tag="psGT")
                    nc.tensor.matmul(psGT[:], xs[64:98, isl], phat[64:98, CS[c]],
                                     start=True, stop=True, tile_position=(64, 0))
                    nc.scalar.sign(ATi[:], psGT[:])
                    nc.tensor.matmul(prodc[0:33, :], xtT[:, i, :], ATi[:],
                                     start=(i == 0), stop=(i == NT - 1),
                                     skip_group_check=True, tile_position=(0, 0))
                    nc.tensor.matmul(prodc[64:97, :], xtT[:, i, :], Ai[:],
                                     start=(i == 0), stop=(i == NT - 1),
                                     skip_group_check=True, tile_position=(0, 64))

                # ---- C(c): uf = (yA'+Sx)/(rs'+N), ub = yAT/cs -> xd rows ----
                nc.scalar.activation(vf[:, CS[c]], prodc[0:33, :], AF.Identity, bias=bSx[:])
                with nc.allow_low_precision(reason="4-byte recips"):
                    nc.vector.reciprocal(rr[:, CS[c]], vf[32:33, CS[c]])
                rrB = pf.tile([T, 512], F32, name="rrB", tag="ps")
                nc.tensor.matmul(rrB[:], ones1[:], rr[:, CS[c]], start=True, stop=True)
                nc.scalar.activation(vb[:, CS[c]], prodc[64:97, :], AF.Identity)
                with nc.allow_low_precision(reason="4-byte recips"):
                    nc.vector.reciprocal(cc[:, CS[c]], vb[32:33, CS[c]])
                nc.vector.tensor_tensor(xd[32:44, CS[c]], vf[0:T, CS[c]], rrB[:], ALU.mult)
                ccB = pf.tile([T, 512], F32, name="ccB", tag="ps")
                nc.tensor.matmul(ccB[:], ones1[:], cc[:, CS[c]], start=True, stop=True)
                nc.vector.tensor_tensor(xd[64:76, CS[c]], vb[0:T, CS[c]], ccB[:], ALU.mult)

                # ---- D(c): x_flat slices ----
                for k in range(6):
                    ps = pf.tile([128, 512], F32, name="psF", tag="ps")
                    nc.tensor.matmul(ps[:], wb[0:76, O_ES + k * 128:O_ES + (k + 1) * 128],
                                     xd[0:76, CS[c]], start=True, stop=True)
                    if k % 2 == 0:
                        nc.scalar.activation(zT[:, k, CS[c]], ps[:], AF.Relu)
                    else:
                        nc.vector.tensor_scalar(zT[:, k, CS[c]], ps[:], 0.0, None, ALU.max)

                # ---- E/F(c): encoder/decoder MLPs ----
                ps = pf.tile([HID2, 512], F32, name="psH1", tag="ps")
                for k in range(6):
                    nc.tensor.matmul(ps[:], wb[:, O_EW1 + k * 128:O_EW1 + (k + 1) * 128],
                                     zT[:, k, CS[c]], start=(k == 0), stop=(k == 5))
                if c % 2 == 0:
                    nc.scalar.activation(h1[:, CS[c]], ps[:], AF.Relu, bias=eb1)
                else:
                    nc.vector.tensor_scalar(h1[:, CS[c]], ps[:], eb1, 0.0, ALU.add, ALU.max)

                ps = pf.tile([HID2, 512], F32, name="psH2", tag="ps")
                nc.tensor.matmul(ps[:], wb[:, O_EW2:O_EW2 + HID2], h1[:, CS[c]],
                                 start=True, stop=True)
                if c % 2 == 0:
                    nc.scalar.activation(h2[:, CS[c]], ps[:], AF.Relu, bias=eb2)
                else:
                    nc.vector.tensor_scalar(h2[:, CS[c]], ps[:], eb2, 0.0, ALU.add, ALU.max)

                ps = pf.tile([H, 512], F32, name="psXe", tag="ps")
                nc.tensor.matmul(ps[:], wb[:, O_EW3:O_EW3 + H], h2[:, CS[c]],
                                 start=True, stop=False)
                for k in range(6):
                    nc.tensor.matmul(ps[:], wb[:, O_EPROJ + k * H:O_EPROJ + (k + 1) * H],
                                     zT[:, k, CS[c]], start=False, stop=(k == 5))
                if c % 2 == 0:
                    nc.scalar.activation(xe[:, CS[c]], ps[:], AF.Identity, bias=ebe)
                else:
                    nc.vector.tensor_scalar(xe[:, CS[c]], ps[:], ebe, None, ALU.add)

                ps = pf.tile([HID2, 512], F32, name="psG1", tag="ps")
                nc.tensor.matmul(ps[:], wb[0:H, O_DW1:O_DW1 + HID2], xe[:, CS[c]],
                                 start=True, stop=True)
                if c % 2 == 1:
                    nc.scalar.activation(g1[:, CS[c]], ps[:], AF.Relu, bias=db1)
                else:
                    nc.vector.tensor_scalar(g1[:, CS[c]], ps[:], db1, 0.0, ALU.add, ALU.max)

                ps = pf.tile([HID2, 512], F32, name="psG2", tag="ps")
                nc.tensor.matmul(ps[:], wb[:, O_DW2:O_DW2 + HID2], g1[:, CS[c]],
                                 start=True, stop=True)
                if c % 2 == 0:
                    nc.scalar.activation(g2[:, CS[c]], ps[:], AF.Relu, bias=db2)
                else:
                    nc.vector.tensor_scalar(g2[:, CS[c]], ps[:], db2, 0.0, ALU.add, ALU.max)

                ps = pf.tile([TOUT, 512], F32, name="psOd", tag="ps")
                nc.tensor.matmul(ps[:], wb[:, O_DW3:O_DW3 + TOUT], g2[:, CS[c]],
                                 start=True, stop=False)
                nc.tensor.matmul(ps[:], wb[0:H, O_DPROJ:O_DPROJ + TOUT], xe[:, CS[c]],
                                 start=False, stop=True)
                if c % 2 == 1:
                    nc.scalar.activation(od[:, CS[c]], ps[:], AF.Identity, bias=dbd)
                else:
                    nc.vector.tensor_scalar(od[:, CS[c]], ps[:], dbd, None, ALU.add)
                eng = (nc.gpsimd, nc.scalar, nc.sync)[c]
                eng.dma_start(out=d["out"].ap()[:, CS[c]], in_=od[:, CS[c]])


def _build_wblob(inputs):
    f32 = np.float32
    W1 = np.asarray(inputs["W1"], f32)[0]
    W2 = np.asarray(inputs["W2"], f32)[0]
    g = np.asarray(inputs["enc_bn_g"], f32); be = np.asarray(inputs["enc_bn_b"], f32)
    m = np.asarray(inputs["enc_bn_m"], f32); v = np.asarray(inputs["enc_bn_v"], f32)
    esc = g / np.sqrt(v + 1e-5)
    ew3 = np.asarray(inputs["enc_w3"], f32) * esc[None, :]
    eproj = np.asarray(inputs["enc_proj"], f32) * esc[None, :]
    ebe = np.asarray(inputs["enc_b3"], f32) * esc + (be - m * esc)
    g = np.asarray(inputs["dec_bn_g"], f32); bd = np.asarray(inputs["dec_bn_b"], f32)
    m = np.asarray(inputs["dec_bn_m"], f32); v = np.asarray(inputs["dec_bn_v"], f32)
    dsc = g / np.sqrt(v + 1e-5)
    dw3 = np.asarray(inputs["dec_w3"], f32) * dsc[None, :]
    dproj = np.asarray(inputs["dec_proj"], f32) * dsc[None, :]
    dbd = np.asarray(inputs["dec_b3"], f32) * dsc + (bd - m * dsc)

    wb = np.zeros((128, CW), f32)
    ew1 = np.asarray(inputs["enc_w1"], f32)
    for a in range(6):
        wb[:, O_EW1 + a * 128:O_EW1 + (a + 1) * 128] = ew1[a * 128:(a + 1) * 128, :]
        wb[:, O_EPROJ + a * H:O_EPROJ + (a + 1) * H] = eproj[a * 128:(a + 1) * 128, :]
    wb[:, O_EW2:O_EW2 + HID2] = np.asarray(inputs["enc_w2"], f32)
    wb[:, O_EW3:O_EW3 + H] = ew3
    wb[0:H, O_DW1:O_DW1 + HID2] = np.asarray(inputs["dec_w1"], f32)
    wb[:, O_DW2:O_DW2 + HID2] = np.asarray(inputs["dec_w2"], f32)
    wb[:, O_DW3:O_DW3 + TOUT] = dw3
    wb[0:H, O_DPROJ:O_DPROJ + TOUT] = dproj
    # [e1t;0;e2a;0;e2b] stack: block-diagonal W rows per t
    for t in range(T):
        wb[t, O_ES + t * H:O_ES + (t + 1) * H] = W1
        wb[32 + t, O_ES + t * H:O_ES + (t + 1) * H] = 0.9 * W2    # K_HOPS * ALPHA
        wb[64 + t, O_ES + t * H:O_ES + (t + 1) * H] = 2.1 * W2    # K_HOPS * (1-ALPHA)
    wb[0:T, O_I12:O_I12 + T] = np.eye(T, dtype=f32)
    wb[:, O_EB1] = np.asarray(inputs["enc_b1"], f32)
    wb[:, O_EB2] = np.asarray(inputs["enc_b2"], f32)
    wb[0:H, O_EBE] = ebe
    wb[:, O_DB1] = np.asarray(inputs["dec_b1"], f32)
    wb[:, O_DB2] = np.asarray(inputs["dec_b2"], f32)
    wb[0:TOUT, O_DBD] = dbd
    return wb


def _weights_fp(inputs):
    """Content fingerprint of every non-x input (cheap; full-content hash)."""
    import hashlib
    h = hashlib.blake2b(digest_size=16)
    for k in sorted(inputs):
        if k == "x":
            continue
        a = np.ascontiguousarray(np.asarray(inputs[k]))
        h.update(k.encode())
        h.update(str(a.shape).encode())
        h.update(a.tobytes())
    return h.digest()


def _make_runner(nc):
    import jax
    from jax.sharding import Mesh, PartitionSpec, NamedSharding
    from jax.experimental.shard_map import shard_map
    from concourse.bass2jax import (_bass_exec_p, install_neuronx_cc_hook,
                                    partition_id_tensor)

    install_neuronx_cc_hook()
    partition_name = nc.partition_id_tensor.name if nc.partition_id_tensor else None

    in_names, out_names, out_avals, zero_shapes = [], [], [], []
    for alloc in nc.m.functions[0].allocations:
        if not isinstance(alloc, mybir.MemoryLocationSet):
            continue
        name = alloc.memorylocations[0].name
        if alloc.kind == "ExternalInput":
            if name != partition_name:
                in_names.append(name)
        elif alloc.kind == "ExternalOutput":
            out_names.append(name)
            shape = tuple(alloc.tensor_shape)
            dtype = mybir.dt.np(alloc.dtype)
            out_avals.append(jax.core.ShapedArray(shape, dtype))
            zero_shapes.append((shape, dtype))
    n_params = len(in_names)
    all_in_names = tuple(in_names + out_names + ([partition_name] if partition_name else []))

    def _body(*args):
        operands = list(args)
        if partition_name is not None:
            operands.append(partition_id_tensor())
        outs = _bass_exec_p.bind(
            *operands,
            out_avals=tuple(out_avals),
            in_names=all_in_names,
            out_names=tuple(out_names),
            lowering_input_output_aliases=(),
            sim_require_finite=True,
            sim_require_nnan=True,
            nc=nc,
        )
        return tuple(outs)

    devices = jax.devices()[:B]
    mesh = Mesh(np.asarray(devices), ("core",))
    nin = n_params + len(out_names)
    sharded = jax.jit(
        shard_map(_body, mesh=mesh, in_specs=(PartitionSpec("core"),) * nin,
                  out_specs=(PartitionSpec("core"),) * len(out_names), check_rep=False),
        keep_unused=True,
    )
    sh = NamedSharding(mesh, PartitionSpec("core"))
    zeros = [jax.device_put(np.zeros((B * s[0], *s[1:]), dt), sh)
             for (s, dt) in zero_shapes]
    return sharded, zeros


def _build_ctx(inputs):
    wb = _build_wblob(inputs)
    nc = _build_nc(wb)
    sharded, zeros = _make_runner(nc)
    return {"fp": _weights_fp(inputs), "nc": nc, "sharded": sharded, "zeros": zeros,
            "ids": tuple(id(inputs[k]) for k in sorted(inputs) if k != "x")}


def kernel(**inputs) -> np.ndarray:
    ctx = _cache.get("ctx")
    if ctx is not None:
        ids = tuple(id(inputs[k]) for k in sorted(inputs) if k != "x")
        if ids != ctx["ids"]:
            if _weights_fp(inputs) == ctx["fp"]:
                ctx["ids"] = ids
            else:
                ctx = None
    if ctx is None:
        ctx = _build_ctx(inputs)
        _cache["ctx"] = ctx

    x = np.asarray(inputs["x"], np.float32).reshape(B * T, N)
    out = ctx["sharded"](x, *ctx["zeros"])[0]
    return np.asarray(out).reshape(B, TOUT, N, 1).astype(np.float32, copy=False)



# revision 22
# speedup vs baseline: 18.0313x; 18.0313x over previous
"""Trainium2 Bass kernel for AdaBiDiff GNN message passing.

Data parallel over batch B=8, one batch element per core.  Per core:
  xt (12,1536) -> softmax over t -> p, logp (t-major)
  kl[i,j] = rowterm[i] - sum_t p[i,t] logp[j,t];  A = (kl < 0.5)
  u_fwd = rownorm(A) @ xt.T;  u_bwd = rownorm(A.T) @ xt.T
  x_flat[n, t*64+h] = relu(xt[t,n] W1[h] + (0.9 u_fwd + 2.1 u_bwd)[t,n] W2[h])
  two MLP blocks (BN folded into weights on host) -> out (12,1536) per core.

Implementation notes:
  - KL adjacency compare computed SCALED by s[i] = Sum_t exp(x[t,i]) > 0:
      s*Ghat[i,j] = Sum_t ex[t,i]x[t,j] + s[i]*(-L[j]) + cmb[i]*1,
    L = ln(s), cmb = (0.5+L)*s - W, W = Sum_t ex*x.  K=34 operand stacks
    (f32r, rows 12-31 zero for the 32-partition alignment rule):
    phat = [ex(12); 0; s@32; cmb@33], xs = [xt(12); 0; -L@32; 1@33].
    No row duplication / explicit tile_position: one plain matmul per
    (i-block, chunk) per orientation.
  - A-orientation (ub side) compare on DVE is_gt -> exact 0/1 fp8e4 tiles.
    AT-orientation (uf side) on ScalarE Sign -> -1/0/1 fp8, with the affine
    correction folded into stage C:  yA = (yS + Sx)/2, rs = (rs' + N)/2
    -> uf = (yS + Sx)/(rs' + N)  (bias column applied on the psum copy).
  - products run as fp8 DoubleRow pair-matmuls (2 i-blocks = K=256 per
    call, 0.5 cyc/col): compare outputs land in [128,2,512] fp8 pair
    stacks; the stationary is a [128,2,48] fp8 transposed-x pack with a
    ones column at 32 (row/col sums land at psum partition 32).
  - reciprocals via reciprocal_approx_fast (18 bits, ~5x faster than
    reciprocal); K=1 matmuls broadcast 1/rs rows to 12 partitions.
  - x_flat is one K=76 matmul per (k,c) against the [W1;0;0.9W2;0;2.1W2]
    block-diagonal stack; MLP data path in f16 (tall moving operands at
    2B/row keep the PE off the SBUF-bandwidth wall).
  - emission order keeps the PE continuously busy (pstate ramp to 2.4GHz
    needs >3us without gaps): psT transposes first, then B(0)C(0) ..
    B(2)C(2) back-to-back, then the MLP tail rounds interleaved across
    the three 512-column chunks.
  - all weights baked into the NEFF as inline consts (f16 blob, one DMA);
    per-call transfers are x in / out back only.  The jitted SPMD
    executable is cached across calls; weight changes detected by
    fingerprint.
"""

import numpy as np

import concourse.bass as bass
import concourse.bacc as bacc
import concourse.tile as tile
import concourse.mybir as mybir

F32 = mybir.dt.float32
F32R = mybir.dt.float32r
F16 = mybir.dt.float16
FP8 = mybir.dt.float8e4
AF = mybir.ActivationFunctionType
ALU = mybir.AluOpType
DR = mybir.MatmulPerfMode.DoubleRow

B, T, N, H, TH, HID2, TOUT = 8, 12, 1536, 64, 768, 128, 12
NT = N // 128          # 12 i-blocks
NP = NT // 2           # 6 DoubleRow pairs
NC = N // 512          # 3 column chunks

# ---- packed f16 weight blob column layout ----
O_ES = 0               # [76, 768] x_flat stack: W1/0.9W2/2.1W2 blockdiag @0/32/64
O_EW1 = 768            # 6 x 128 cols, rows 0-127
O_EPROJ = 1536         # 6 x 64 cols, rows 0-127
O_EW2 = 1920           # 128 cols
O_EW3 = 2048           # 64 cols
O_DW1 = 2112           # 128 cols, rows 0-63
O_DW2 = 2240           # 128 cols
O_DW3 = 2368           # 12 cols
O_DPROJ = 2380         # 12 cols, rows 0-63
O_EB1 = 2392           # f32 bias columns (pairs of f16 cols, bitcast)
O_EB2 = 2394
O_EBE = 2396
O_DB1 = 2398
O_DB2 = 2400
O_DBD = 2402
CW = 2404

_cache = {}


def _build_nc(wblob):
    nc = bacc.Bacc("TRN2", target_bir_lowering=False, debug=False)
    d = {}
    d["x"] = nc.declare_dram_parameter("x", [T, N], F32, isOutput=False)
    d["out"] = nc.declare_dram_parameter("out", [T, N], F32, isOutput=True)
    d["wb"] = nc.inline_tensor(wblob, name="wb")
    d["i12"] = nc.inline_tensor(np.eye(T, dtype=np.float16), name="i12")
    # zeros rows 0-20, ones row 21: one blob serves xs[12:34] (zeros + ones
    # row at 33) and phat[12:32] (zeros)
    zc = np.zeros((22, N), np.float16)
    zc[21, :] = 1.0
    d["zc"] = nc.inline_tensor(zc, name="zc")

    with tile.TileContext(nc) as tc:
        _kernel_body(tc, d)
    nc.compile()
    return nc


def _kernel_body(tc, d):
    nc = tc.nc
    CS = [slice(c * 512, (c + 1) * 512) for c in range(NC)]

    with tc.tile_pool(name="w", bufs=1) as w, tc.tile_pool(name="sb", bufs=1) as sb:

        def stile(name, shape, dt):
            return sb.tile(list(shape), dt, name=name, tag=name)

        # ---- per-call input + consts ----
        xin = stile("xin", (T, N), F32)
        nc.sync.dma_start(out=xin[:], in_=d["x"].ap())
        # xs: Ghat j-side stack [xt(12); 0(20); -L@32; 1@33] f16
        xs = stile("xs", (34, N), F16)
        nc.gpsimd.dma_start(out=xs[T:34, :], in_=d["zc"].ap())
        wb = w.tile([128, CW], F16, name="wb", tag="wb")
        nc.scalar.dma_start(out=wb[:], in_=d["wb"].ap())
        i12 = w.tile([T, T], F16, name="i12", tag="i12")
        nc.gpsimd.dma_start(out=i12[:], in_=d["i12"].ap())
        # phat: Ghat i-side stack [ex(12); 0(20); s@32; cmb@33] f16
        phat = stile("phat", (34, N), F16)
        nc.gpsimd.dma_start(out=phat[T:32, :], in_=d["zc"].ap()[0:20, :])

        ones12 = w.tile([T, 1], F16, name="ones12", tag="ones12")
        nc.vector.memset(ones12[:], 1.0)
        ones1 = w.tile([1, T], F32R, name="ones1", tag="ones1")
        nc.vector.memset(ones1[:].bitcast(F32), 1.0)
        # uf affine-correction bias: rows 0-11 = Sum_i xt[t,i], row 32 = N
        bSx = w.tile([33, 1], F32, name="bSx", tag="bSx")
        nc.vector.memset(bSx[32:33, :], float(N))
        # prewarm exp table under the input DMA
        warm = w.tile([1, 1], F32, name="warm", tag="warm")
        nc.vector.memset(warm[:], 1.0)
        nc.scalar.activation(warm[:], warm[:], AF.Exp)

        bias = {
            "eb1": wb[:, O_EB1:O_EB1 + 2].bitcast(F32),
            "eb2": wb[:, O_EB2:O_EB2 + 2].bitcast(F32),
            "ebe": wb[0:H, O_EBE:O_EBE + 2].bitcast(F32),
            "db1": wb[:, O_DB1:O_DB1 + 2].bitcast(F32),
            "db2": wb[:, O_DB2:O_DB2 + 2].bitcast(F32),
            "dbd": wb[0:TOUT, O_DBD:O_DBD + 2].bitcast(F32),
        }

        # fp8 transposed-x pair stack: [p, pair, member, col] col 32 = ones
        xtT = stile("xtT", (128, NP, 2, 48), FP8)
        nc.gpsimd.memset(xtT[:], 0.0)
        nc.vector.memset(xtT[:, :, :, 32:33], 1.0)
        # x_flat moving stack [xt(12); 0; uf@32; 0; ub@64] f16
        xd = stile("xd", (76, N), F16)
        nc.gpsimd.memset(xd[:], 0.0)
        nc.vector.tensor_copy(xs[0:T, :], xin[:])
        nc.vector.tensor_copy(xd[0:T, :], xs[0:T, :])

        # =========== Stage A ===========
        xt = xs[0:T, :]
        wx = stile("wx", (T, N), F16)
        cm32 = stile("cm32", (1, N), F32)
        with tc.tile_pool(name="pa", bufs=1, space="PSUM") as pa:
            nc.scalar.activation(phat[0:T, :], xin[:], AF.Exp)
            nc.scalar.activation(warm[:], warm[:], AF.Ln)   # hide Ln table load
            nc.vector.tensor_tensor(wx[:], phat[0:T, :], xt, ALU.mult)
            nc.vector.tensor_reduce(bSx[0:T, :], xt,
                                    mybir.AxisListType.X, ALU.add)

            # s sums first: they gate the serial aug chain, which then runs
            # under the psW/psT matmuls
            psA = pa.tile([1, NC, 512], F32, name="psA", tag="psA")
            psW = pa.tile([1, NC, 512], F32, name="psW", tag="psW")
            for c in range(NC):
                nc.tensor.matmul(psA[:, c, :], ones12[:], phat[0:T, CS[c]],
                                 start=True, stop=True)
            for c in range(NC):
                nc.tensor.matmul(psW[:, c, :], ones12[:], wx[:, CS[c]],
                                 start=True, stop=True)
            del wx

            psT = pa.tile([128, NT, T], F32, name="psT", tag="psT")
            for j in range(NT):
                nc.tensor.matmul(psT[:, j, :], xs[0:T, j * 128:(j + 1) * 128],
                                 i12[:], start=True, stop=True)
            for j in range(NT):
                nc.vector.tensor_copy(xtT[:, j // 2, j % 2, 0:T], psT[:, j, :])

            # aug chain, chunked so B(0)'s first i-blocks unblock on chunk-0
            # aug only.  All Ln ops adjacent (no act-table thrash); -L / s /
            # cmb on DVE; augP row 0 = s, row 32 = cmb (f16); per-chunk
            # strided DMA -> phat rows 32-33
            L = stile("L", (1, N), F32)
            augP = stile("augP", (33, N), F16)
            for c in range(NC):
                nc.scalar.activation(L[:, CS[c]], psA[:, c, :], AF.Ln)
            for c in range(NC):
                nc.vector.tensor_scalar(xs[32:33, CS[c]], L[:, CS[c]], -1.0,
                                        None, ALU.mult)
                nc.vector.tensor_copy(augP[0:1, CS[c]], psA[:, c, :])
                # cmb = (0.5 + L)*s - W, f32 intermediates, f16 final
                nc.vector.tensor_scalar(cm32[:, CS[c]], L[:, CS[c]], 0.5, None,
                                        ALU.add)
                nc.vector.tensor_tensor(cm32[:, CS[c]], cm32[:, CS[c]],
                                        psA[:, c, :], ALU.mult)
                nc.vector.tensor_tensor(augP[32:33, CS[c]], cm32[:, CS[c]],
                                        psW[:, c, :], ALU.subtract)
                eng = (nc.sync, nc.gpsimd, nc.scalar)[c]
                eng.dma_start(out=phat[32:34, CS[c]], in_=augP[0:33:32, CS[c]])

        # =========== Stages B/C then MLP tail ===========
        vf = stile("vf", (T, N), F32)
        vb = stile("vb", (T, N), F32)
        rrA = stile("rrA", (1, N), F32R)
        rrB = stile("rrB", (1, N), F32R)
        zT = stile("zT", (128, 6, N), F16)
        h1 = stile("h1", (HID2, N), F16)
        h2 = stile("h2", (HID2, N), F16)
        xe = stile("xe", (H, N), F16)
        g1 = stile("g1", (HID2, N), F16)
        g2 = stile("g2", (HID2, N), F16)
        od = stile("od", (TOUT, N), F32)

        with tc.tile_pool(name="pG", bufs=3, space="PSUM") as pG, \
             tc.tile_pool(name="pPA", bufs=1, space="PSUM") as pPA, \
             tc.tile_pool(name="pPB", bufs=1, space="PSUM") as pPB, \
             tc.tile_pool(name="ab", bufs=2) as ab, \
             tc.tile_pool(name="pf", bufs=3, space="PSUM") as pf:

            prodq = []       # pending pair products: (pA, pB, Aq, Sq, q, last)
            c2q = []         # deferred C2 stages: (c, pA?, ...) emitted later

            def emit_products():
                if not prodq:
                    return
                pA, pB, Aq, Sq, q, last = prodq.pop(0)
                nc.tensor.matmul(pA[:], xtT[:, q], Aq[:],
                                 start=(q == 0), stop=last, perf_mode=DR)
                nc.tensor.matmul(pB[:], xtT[:, q], Sq[:],
                                 start=(q == 0), stop=last, perf_mode=DR)

            def emit_c2(c):
                # uf/ub broadcast matmuls + xd multiplies (PE + DVE);
                # the recip inputs are long done when these are emitted
                uB = pf.tile([T, 512], F32, name="uB", tag="ps")
                nc.tensor.matmul(uB[:], ones1[:], rrB[:, CS[c]],
                                 start=True, stop=True)
                nc.vector.tensor_tensor(xd[32:44, CS[c]], vf[:, CS[c]],
                                        uB[:], ALU.mult)
                uA = pf.tile([T, 512], F32, name="uA", tag="ps")
                nc.tensor.matmul(uA[:], ones1[:], rrA[:, CS[c]],
                                 start=True, stop=True)
                nc.vector.tensor_tensor(xd[64:76, CS[c]], vb[:, CS[c]],
                                        uA[:], ALU.mult)

            for c in range(NC):
                # ---- B(c): Ghat, compares; pair products lag one pair ----
                pA = pPA.tile([48, 512], F32, name="pA", tag="pA")
                pB = pPB.tile([48, 512], F32, name="pB", tag="pB")
                for q in range(NP):
                    Aq = ab.tile([128, 2, 512], FP8, name="Aq", tag="Aq")
                    Sq = ab.tile([128, 2, 512], FP8, name="Sq", tag="Sq")
                    for m in range(2):
                        isl = slice((2 * q + m) * 128, (2 * q + m + 1) * 128)
                        psG = pG.tile([128, 512], F32, name="psG", tag="g")
                        nc.tensor.matmul(psG[:], phat[:, isl], xs[:, CS[c]],
                                         start=True, stop=True)
                        nc.vector.tensor_scalar(Aq[:, m, :], psG[:], 0.0, None,
                                                ALU.is_gt)
                        psGT = pG.tile([128, 512], F32, name="psGT", tag="g")
                        nc.tensor.matmul(psGT[:], xs[:, isl], phat[:, CS[c]],
                                         start=True, stop=True)
                        nc.scalar.sign(Sq[:, m, :], psGT[:])
                    emit_products()
                    prodq.append((pA, pB, Aq, Sq, q, q == NP - 1))
                    if q == 0 and c2q:
                        emit_c2(c2q.pop(0))
                emit_products()

                # ---- C1(c): psum copies + reciprocals (no PE) ----
                nc.scalar.activation(vb[:, CS[c]], pA[0:T, :], AF.Identity)
                nc.scalar.activation(rrA[:, CS[c]], pA[32:33, :], AF.Identity)
                nc.scalar.activation(vf[:, CS[c]], pB[0:T, :], AF.Identity,
                                     bias=bSx[0:T, :])
                nc.scalar.activation(rrB[:, CS[c]], pB[32:33, :],
                                     AF.Identity, bias=bSx[32:33, :])
                from concourse.dve_ops import (RECIP_APPROX_FAST_CONSTS as RC,
                                               RECIPROCAL_APPROX_FAST as RAF)
                nc.vector._custom_dve(RAF, out=rrA[:, CS[c]], in0=rrA[:, CS[c]],
                                      s0=RC["s0"], s1=RC["s1"], imm2=RC["imm2"])
                nc.vector._custom_dve(RAF, out=rrB[:, CS[c]], in0=rrB[:, CS[c]],
                                      s0=RC["s0"], s1=RC["s1"], imm2=RC["imm2"])
                c2q.append(c)

            # ---- D rounds for chunks 0..1, then C2(2), then D(2) ----
            for k in range(6):
                for c in range(NC - 1):
                    ps = pf.tile([128, 512], F32, name="psF", tag="ps")
                    nc.tensor.matmul(ps[:], wb[0:76, O_ES + k * 128:O_ES + (k + 1) * 128],
                                     xd[:, CS[c]], start=True, stop=True)
                    if (k + c) % 2 == 0:
                        nc.scalar.activation(zT[:, k, CS[c]], ps[:], AF.Relu)
                    else:
                        nc.vector.tensor_scalar(zT[:, k, CS[c]], ps[:], 0.0,
                                                None, ALU.max)
                if k == 1 and c2q:
                    emit_c2(c2q.pop(0))
            for k in range(6):
                c = NC - 1
                ps = pf.tile([128, 512], F32, name="psF", tag="ps")
                nc.tensor.matmul(ps[:], wb[0:76, O_ES + k * 128:O_ES + (k + 1) * 128],
                                 xd[:, CS[c]], start=True, stop=True)
                if (k + c) % 2 == 0:
                    nc.scalar.activation(zT[:, k, CS[c]], ps[:], AF.Relu)
                else:
                    nc.vector.tensor_scalar(zT[:, k, CS[c]], ps[:], 0.0,
                                            None, ALU.max)

            # ---- MLP tail rounds ----
            for c in range(NC):
                ps = pf.tile([HID2, 512], F32, name="psH1", tag="ps")
                for k in range(6):
                    nc.tensor.matmul(ps[:], wb[:, O_EW1 + k * 128:O_EW1 + (k + 1) * 128],
                                     zT[:, k, CS[c]], start=(k == 0), stop=(k == 5))
                if c % 2 == 0:
                    nc.scalar.activation(h1[:, CS[c]], ps[:], AF.Relu, bias=bias["eb1"])
                else:
                    nc.vector.tensor_scalar(h1[:, CS[c]], ps[:], bias["eb1"], 0.0,
                                            ALU.add, ALU.max)

            for c in range(NC):
                ps = pf.tile([HID2, 512], F32, name="psH2", tag="ps")
                nc.tensor.matmul(ps[:], wb[:, O_EW2:O_EW2 + HID2], h1[:, CS[c]],
                                 start=True, stop=True)
                if c % 2 == 1:
                    nc.scalar.activation(h2[:, CS[c]], ps[:], AF.Relu, bias=bias["eb2"])
                else:
                    nc.vector.tensor_scalar(h2[:, CS[c]], ps[:], bias["eb2"], 0.0,
                                            ALU.add, ALU.max)

            for c in range(NC):
                ps = pf.tile([H, 512], F32, name="psXe", tag="ps")
                nc.tensor.matmul(ps[:], wb[:, O_EW3:O_EW3 + H], h2[:, CS[c]],
                                 start=True, stop=False)
                for k in range(6):
                    nc.tensor.matmul(ps[:], wb[:, O_EPROJ + k * H:O_EPROJ + (k + 1) * H],
                                     zT[:, k, CS[c]], start=False, stop=(k == 5))
                if c % 2 == 0:
                    nc.scalar.activation(xe[:, CS[c]], ps[:], AF.Identity, bias=bias["ebe"])
                else:
                    nc.vector.tensor_scalar(xe[:, CS[c]], ps[:], bias["ebe"], None,
                                            ALU.add)

            for c in range(NC):
                ps = pf.tile([HID2, 512], F32, name="psG1", tag="ps")
                nc.tensor.matmul(ps[:], wb[0:H, O_DW1:O_DW1 + HID2], xe[:, CS[c]],
                                 start=True, stop=True)
                if c % 2 == 1:
                    nc.scalar.activation(g1[:, CS[c]], ps[:], AF.Relu, bias=bias["db1"])
                else:
                    nc.vector.tensor_scalar(g1[:, CS[c]], ps[:], bias["db1"], 0.0,
                                            ALU.add, ALU.max)

            for c in range(NC):
                ps = pf.tile([HID2, 512], F32, name="psG2", tag="ps")
                nc.tensor.matmul(ps[:], wb[:, O_DW2:O_DW2 + HID2], g1[:, CS[c]],
                                 start=True, stop=True)
                if c % 2 == 0:
                    nc.scalar.activation(g2[:, CS[c]], ps[:], AF.Relu, bias=bias["db2"])
                else:
                    nc.vector.tensor_scalar(g2[:, CS[c]], ps[:], bias["db2"], 0.0,
                                            ALU.add, ALU.max)

            for c in range(NC):
                ps = pf.tile([TOUT, 512], F32, name="psOd", tag="ps")
                nc.tensor.matmul(ps[:], wb[:, O_DW3:O_DW3 + TOUT], g2[:, CS[c]],
                                 start=True, stop=False)
                nc.tensor.matmul(ps[:], wb[0:H, O_DPROJ:O_DPROJ + TOUT], xe[:, CS[c]],
                                 start=False, stop=True)
                if c % 2 == 1:
                    nc.scalar.activation(od[:, CS[c]], ps[:], AF.Identity, bias=bias["dbd"])
                else:
                    nc.vector.tensor_scalar(od[:, CS[c]], ps[:], bias["dbd"], None,
                                            ALU.add)
                eng = (nc.gpsimd, nc.scalar, nc.sync)[c]
                eng.dma_start(out=d["out"].ap()[:, CS[c]], in_=od[:, CS[c]])


def _build_wblob(inputs):
    f32 = np.float32
    f16 = np.float16
    W1 = np.asarray(inputs["W1"], f32)[0]
    W2 = np.asarray(inputs["W2"], f32)[0]
    g = np.asarray(inputs["enc_bn_g"], f32); be = np.asarray(inputs["enc_bn_b"], f32)
    m = np.asarray(inputs["enc_bn_m"], f32); v = np.asarray(inputs["enc_bn_v"], f32)
    esc = g / np.sqrt(v + 1e-5)
    ew3 = np.asarray(inputs["enc_w3"], f32) * esc[None, :]
    eproj = np.asarray(inputs["enc_proj"], f32) * esc[None, :]
    ebe = np.asarray(inputs["enc_b3"], f32) * esc + (be - m * esc)
    g = np.asarray(inputs["dec_bn_g"], f32); bd = np.asarray(inputs["dec_bn_b"], f32)
    m = np.asarray(inputs["dec_bn_m"], f32); v = np.asarray(inputs["dec_bn_v"], f32)
    dsc = g / np.sqrt(v + 1e-5)
    dw3 = np.asarray(inputs["dec_w3"], f32) * dsc[None, :]
    dproj = np.asarray(inputs["dec_proj"], f32) * dsc[None, :]
    dbd = np.asarray(inputs["dec_b3"], f32) * dsc + (bd - m * dsc)

    wb = np.zeros((128, CW), f16)
    # x_flat stack: block-diagonal per t at rows 0/32/64
    for t in range(T):
        wb[t, O_ES + t * H:O_ES + (t + 1) * H] = W1.astype(f16)
        wb[32 + t, O_ES + t * H:O_ES + (t + 1) * H] = (0.9 * W2).astype(f16)
        wb[64 + t, O_ES + t * H:O_ES + (t + 1) * H] = (2.1 * W2).astype(f16)
    ew1 = np.asarray(inputs["enc_w1"], f32)
    for a in range(6):
        wb[:, O_EW1 + a * 128:O_EW1 + (a + 1) * 128] = ew1[a * 128:(a + 1) * 128, :].astype(f16)
        wb[:, O_EPROJ + a * H:O_EPROJ + (a + 1) * H] = eproj[a * 128:(a + 1) * 128, :].astype(f16)
    wb[:, O_EW2:O_EW2 + HID2] = np.asarray(inputs["enc_w2"], f32).astype(f16)
    wb[:, O_EW3:O_EW3 + H] = ew3.astype(f16)
    wb[0:H, O_DW1:O_DW1 + HID2] = np.asarray(inputs["dec_w1"], f32).astype(f16)
    wb[:, O_DW2:O_DW2 + HID2] = np.asarray(inputs["dec_w2"], f32).astype(f16)
    wb[:, O_DW3:O_DW3 + TOUT] = dw3.astype(f16)
    wb[0:H, O_DPROJ:O_DPROJ + TOUT] = dproj.astype(f16)

    def put_f32_col(off, rows, vals):
        col = np.zeros(128, f32)
        col[:rows] = vals
        wb[:, off:off + 2] = col.view(f16).reshape(128, 2)

    put_f32_col(O_EB1, HID2, np.asarray(inputs["enc_b1"], f32))
    put_f32_col(O_EB2, HID2, np.asarray(inputs["enc_b2"], f32))
    put_f32_col(O_EBE, H, ebe)
    put_f32_col(O_DB1, HID2, np.asarray(inputs["dec_b1"], f32))
    put_f32_col(O_DB2, HID2, np.asarray(inputs["dec_b2"], f32))
    put_f32_col(O_DBD, TOUT, dbd)
    return wb


def _weights_fp(inputs):
    """Content fingerprint of every non-x input (cheap; full-content hash)."""
    import hashlib
    h = hashlib.blake2b(digest_size=16)
    for k in sorted(inputs):
        if k == "x":
            continue
        a = np.ascontiguousarray(np.asarray(inputs[k]))
        h.update(k.encode())
        h.update(str(a.shape).encode())
        h.update(a.tobytes())
    return h.digest()


def _make_runner(nc):
    import jax
    from jax.sharding import Mesh, PartitionSpec, NamedSharding
    from jax.experimental.shard_map import shard_map
    from concourse.bass2jax import (_bass_exec_p, install_neuronx_cc_hook,
                                    partition_id_tensor)

    install_neuronx_cc_hook()
    partition_name = nc.partition_id_tensor.name if nc.partition_id_tensor else None

    in_names, out_names, out_avals, zero_shapes = [], [], [], []
    for alloc in nc.m.functions[0].allocations:
        if not isinstance(alloc, mybir.MemoryLocationSet):
            continue
        name = alloc.memorylocations[0].name
        if alloc.kind == "ExternalInput":
            if name != partition_name:
                in_names.append(name)
        elif alloc.kind == "ExternalOutput":
            out_names.append(name)
            shape = tuple(alloc.tensor_shape)
            dtype = mybir.dt.np(alloc.dtype)
            out_avals.append(jax.core.ShapedArray(shape, dtype))
            zero_shapes.append((shape, dtype))
    n_params = len(in_names)
    all_in_names = tuple(in_names + out_names + ([partition_name] if partition_name else []))

    def _body(*args):
        operands = list(args)
        if partition_name is not None:
            operands.append(partition_id_tensor())
        outs = _bass_exec_p.bind(
            *operands,
            out_avals=tuple(out_avals),
            in_names=all_in_names,
            out_names=tuple(out_names),
            lowering_input_output_aliases=(),
            sim_require_finite=True,
            sim_require_nnan=True,
            nc=nc,
        )
        return tuple(outs)

    devices = jax.devices()[:B]
    mesh = Mesh(np.asarray(devices), ("core",))
    nin = n_params + len(out_names)
    sharded = jax.jit(
        shard_map(_body, mesh=mesh, in_specs=(PartitionSpec("core"),) * nin,
                  out_specs=(PartitionSpec("core"),) * len(out_names), check_rep=False),
        keep_unused=True,
    )
    sh = NamedSharding(mesh, PartitionSpec("core"))
    zeros = [jax.device_put(np.zeros((B * s[0], *s[1:]), dt), sh)
             for (s, dt) in zero_shapes]
    return sharded, zeros


def _build_ctx(inputs):
    wb = _build_wblob(inputs)
    nc = _build_nc(wb)
    sharded, zeros = _make_runner(nc)
    return {"fp": _weights_fp(inputs), "nc": nc, "sharded": sharded, "zeros": zeros,
            "ids": tuple(id(inputs[k]) for k in sorted(inputs) if k != "x")}


def kernel(**inputs) -> np.ndarray:
    ctx = _cache.get("ctx")
    if ctx is not None:
        ids = tuple(id(inputs[k]) for k in sorted(inputs) if k != "x")
        if ids != ctx["ids"]:
            if _weights_fp(inputs) == ctx["fp"]:
                ctx["ids"] = ids
            else:
                ctx = None
    if ctx is None:
        ctx = _build_ctx(inputs)
        _cache["ctx"] = ctx

    x = np.asarray(inputs["x"], np.float32).reshape(B * T, N)
    out = ctx["sharded"](x, *ctx["zeros"])[0]
    return np.asarray(out).reshape(B, TOUT, N, 1).astype(np.float32, copy=False)


# revision 26
# speedup vs baseline: 18.3138x; 1.0157x over previous
"""Trainium2 Bass kernel for AdaBiDiff GNN message passing.

Data parallel over batch B=8, one batch element per core.  Per core:
  xt (12,1536) -> softmax over t -> p, logp (t-major)
  kl[i,j] = rowterm[i] - sum_t p[i,t] logp[j,t];  A = (kl < 0.5)
  u_fwd = rownorm(A) @ xt.T;  u_bwd = rownorm(A.T) @ xt.T
  x_flat[n, t*64+h] = relu(xt[t,n] W1[h] + (0.9 u_fwd + 2.1 u_bwd)[t,n] W2[h])
  two MLP blocks (BN folded into weights on host) -> out (12,1536) per core.

Implementation notes:
  - KL adjacency compare computed SCALED by s[i] = Sum_t exp(x[t,i]) > 0:
      s*Ghat[i,j] = Sum_t ex[t,i]x[t,j] + s[i]*(-L[j]) + cmb[i]*1,
    L = ln(s), cmb = (0.5+L)*s - W, W = Sum_t ex*x.  K=34 operand stacks
    (f32r, rows 12-31 zero for the 32-partition alignment rule):
    phat = [ex(12); 0; s@32; cmb@33], xs = [xt(12); 0; -L@32; 1@33].
    No row duplication / explicit tile_position: one plain matmul per
    (i-block, chunk) per orientation.
  - A-orientation (ub side) compare on DVE is_gt -> exact 0/1 fp8e4 tiles.
    AT-orientation (uf side) on ScalarE Sign -> -1/0/1 fp8, with the affine
    correction folded into stage C:  yA = (yS + Sx)/2, rs = (rs' + N)/2
    -> uf = (yS + Sx)/(rs' + N)  (bias column applied on the psum copy).
  - products run as fp8 DoubleRow pair-matmuls (2 i-blocks = K=256 per
    call, 0.5 cyc/col): compare outputs land in [128,2,512] fp8 pair
    stacks; the stationary is a [128,2,48] fp8 transposed-x pack with a
    ones column at 32 (row/col sums land at psum partition 32).
  - reciprocals via reciprocal_approx_fast (18 bits, ~5x faster than
    reciprocal); K=1 matmuls broadcast 1/rs rows to 12 partitions.
  - x_flat is one K=76 matmul per (k,c) against the [W1;0;0.9W2;0;2.1W2]
    block-diagonal stack; MLP data path in f16 (tall moving operands at
    2B/row keep the PE off the SBUF-bandwidth wall).
  - emission order keeps the PE continuously busy (pstate ramp to 2.4GHz
    needs >3us without gaps): psT transposes first, then B(0)C(0) ..
    B(2)C(2) back-to-back, then the MLP tail rounds interleaved across
    the three 512-column chunks.
  - all weights baked into the NEFF as inline consts (f16 blob, one DMA);
    per-call transfers are x in / out back only.  The jitted SPMD
    executable is cached across calls; weight changes detected by
    fingerprint.
"""

import numpy as np

import concourse.bass as bass
import concourse.bacc as bacc
import concourse.tile as tile
import concourse.mybir as mybir

F32 = mybir.dt.float32
F32R = mybir.dt.float32r
F16 = mybir.dt.float16
FP8 = mybir.dt.float8e4
AF = mybir.ActivationFunctionType
ALU = mybir.AluOpType
DR = mybir.MatmulPerfMode.DoubleRow

B, T, N, H, TH, HID2, TOUT = 8, 12, 1536, 64, 768, 128, 12
NT = N // 128          # 12 i-blocks
NP = NT // 2           # 6 DoubleRow pairs
NC = N // 512          # 3 column chunks

# ---- packed f16 weight blob column layout ----
O_ES = 0               # [76, 768] x_flat stack: W1/0.9W2/2.1W2 blockdiag @0/32/64
O_EW1 = 768            # 6 x 128 cols, rows 0-127
O_EPROJ = 1536         # 6 x 64 cols, rows 0-127
O_EW2 = 1920           # 128 cols
O_EW3 = 2048           # 64 cols
O_DW1 = 2112           # 128 cols, rows 0-63
O_DW2 = 2240           # 128 cols
O_DW3 = 2368           # 12 cols
O_DPROJ = 2380         # 12 cols, rows 0-63
O_EB1 = 2392           # f32 bias columns (pairs of f16 cols, bitcast)
O_EB2 = 2394
O_EBE = 2396
O_DB1 = 2398
O_DB2 = 2400
O_DBD = 2402
CW = 2404

_cache = {}


def _build_nc(wblob):
    nc = bacc.Bacc("TRN2", target_bir_lowering=False, debug=False)
    d = {}
    d["x"] = nc.declare_dram_parameter("x", [T, N], F32, isOutput=False)
    d["out"] = nc.declare_dram_parameter("out", [T, N], F32, isOutput=True)
    d["wb"] = nc.inline_tensor(wblob, name="wb")
    d["i12"] = nc.inline_tensor(np.eye(T, dtype=np.float16), name="i12")
    # zeros rows 0-20, ones row 21: one blob serves xs[12:34] (zeros + ones
    # row at 33) and phat[12:32] (zeros)
    zc = np.zeros((22, N), np.float16)
    zc[21, :] = 1.0
    d["zc"] = nc.inline_tensor(zc, name="zc")

    with tile.TileContext(nc) as tc:
        _kernel_body(tc, d)
    nc.compile()
    return nc


def _kernel_body(tc, d):
    nc = tc.nc
    CS = [slice(c * 512, (c + 1) * 512) for c in range(NC)]

    with tc.tile_pool(name="w", bufs=1) as w, tc.tile_pool(name="sb", bufs=1) as sb:

        def stile(name, shape, dt):
            return sb.tile(list(shape), dt, name=name, tag=name)

        # ---- per-call input + consts ----
        xin = stile("xin", (T, N), F32)
        nc.sync.dma_start(out=xin[:], in_=d["x"].ap())
        # xs: Ghat j-side stack [xt(12); 0(20); -L@32; 1@33] f16
        xs = stile("xs", (34, N), F16)
        nc.gpsimd.dma_start(out=xs[T:34, :], in_=d["zc"].ap())
        wb = w.tile([128, CW], F16, name="wb", tag="wb")
        nc.scalar.dma_start(out=wb[:], in_=d["wb"].ap())
        i12 = w.tile([T, T], F16, name="i12", tag="i12")
        nc.gpsimd.dma_start(out=i12[:], in_=d["i12"].ap())
        # phat: Ghat i-side stack [ex(12); 0(20); s@32; cmb@33] f16
        phat = stile("phat", (34, N), F16)
        nc.gpsimd.dma_start(out=phat[T:32, :], in_=d["zc"].ap()[0:20, :])

        ones12 = w.tile([T, 1], F16, name="ones12", tag="ones12")
        nc.vector.memset(ones12[:], 1.0)
        ones1 = w.tile([1, T], F32R, name="ones1", tag="ones1")
        nc.vector.memset(ones1[:].bitcast(F32), 1.0)
        # uf affine-correction bias: rows 0-11 = Sum_i xt[t,i], row 32 = N
        bSx = w.tile([33, 1], F32, name="bSx", tag="bSx")
        nc.vector.memset(bSx[32:33, :], float(N))
        # prewarm exp table under the input DMA
        warm = w.tile([1, 1], F32, name="warm", tag="warm")
        nc.vector.memset(warm[:], 1.0)
        nc.scalar.activation(warm[:], warm[:], AF.Exp)

        bias = {
            "eb1": wb[:, O_EB1:O_EB1 + 2].bitcast(F32),
            "eb2": wb[:, O_EB2:O_EB2 + 2].bitcast(F32),
            "ebe": wb[0:H, O_EBE:O_EBE + 2].bitcast(F32),
            "db1": wb[:, O_DB1:O_DB1 + 2].bitcast(F32),
            "db2": wb[:, O_DB2:O_DB2 + 2].bitcast(F32),
            "dbd": wb[0:TOUT, O_DBD:O_DBD + 2].bitcast(F32),
        }

        # fp8 transposed-x pair stack: [p, pair, member, col] col 32 = ones
        xtT = stile("xtT", (128, NP, 2, 48), FP8)
        nc.gpsimd.memset(xtT[:], 0.0)
        nc.vector.memset(xtT[:, :, :, 32:33], 1.0)
        # x_flat moving stack [xt(12); 0; uf@32; 0; ub@64] f16
        xd = stile("xd", (76, N), F16)
        nc.gpsimd.memset(xd[:], 0.0)
        nc.vector.tensor_copy(xs[0:T, :], xin[:])
        nc.vector.tensor_copy(xd[0:T, :], xs[0:T, :])

        # =========== Stage A ===========
        xt = xs[0:T, :]
        wx = stile("wx", (T, N), F16)
        cm32 = stile("cm32", (1, N), F32)
        with tc.tile_pool(name="pa", bufs=1, space="PSUM") as pa:
            nc.scalar.activation(phat[0:T, :], xin[:], AF.Exp)
            nc.scalar.activation(warm[:], warm[:], AF.Ln)   # hide Ln table load
            nc.vector.tensor_tensor(wx[:], phat[0:T, :], xt, ALU.mult)
            nc.vector.tensor_reduce(bSx[0:T, :], xt,
                                    mybir.AxisListType.X, ALU.add)

            # s sums first: they gate the serial aug chain, which then runs
            # under the psW/psT matmuls
            psA = pa.tile([1, NC, 512], F32, name="psA", tag="psA")
            psW = pa.tile([1, NC, 512], F32, name="psW", tag="psW")
            for c in range(NC):
                nc.tensor.matmul(psA[:, c, :], ones12[:], phat[0:T, CS[c]],
                                 start=True, stop=True)
            for c in range(NC):
                nc.tensor.matmul(psW[:, c, :], ones12[:], wx[:, CS[c]],
                                 start=True, stop=True)
            del wx

            psT = pa.tile([128, NT, T], F32, name="psT", tag="psT")
            for j in range(NT):
                nc.tensor.matmul(psT[:, j, :], xs[0:T, j * 128:(j + 1) * 128],
                                 i12[:], start=True, stop=True)
            for j in range(NT):
                nc.vector.tensor_copy(xtT[:, j // 2, j % 2, 0:T], psT[:, j, :])

            # aug chain, chunked so B(0)'s first i-blocks unblock on chunk-0
            # aug only.  All Ln ops adjacent (no act-table thrash); -L / s /
            # cmb on DVE; augP row 0 = s, row 32 = cmb (f16); per-chunk
            # strided DMA -> phat rows 32-33
            L = stile("L", (1, N), F32)
            augP = stile("augP", (33, N), F16)
            for c in range(NC):
                nc.scalar.activation(L[:, CS[c]], psA[:, c, :], AF.Ln)
            for c in range(NC):
                nc.vector.tensor_scalar(xs[32:33, CS[c]], L[:, CS[c]], -1.0,
                                        None, ALU.mult)
                nc.vector.tensor_copy(augP[0:1, CS[c]], psA[:, c, :])
                # cmb = (0.5 + L)*s - W, f32 intermediates, f16 final
                nc.vector.scalar_tensor_tensor(cm32[:, CS[c]], L[:, CS[c]], 0.5,
                                               psA[:, c, :], ALU.add, ALU.mult)
                nc.vector.tensor_tensor(augP[32:33, CS[c]], cm32[:, CS[c]],
                                        psW[:, c, :], ALU.subtract)
                eng = (nc.sync, nc.gpsimd, nc.scalar)[c]
                eng.dma_start(out=phat[32:34, CS[c]], in_=augP[0:33:32, CS[c]])

        # =========== Stages B/C then MLP tail ===========
        vf = stile("vf", (T, N), F32)
        vb = stile("vb", (T, N), F32)
        rrA = stile("rrA", (1, N), F32R)
        rrB = stile("rrB", (1, N), F32R)
        zT = stile("zT", (128, 6, N), F16)
        h1 = stile("h1", (HID2, N), F16)
        h2 = stile("h2", (HID2, N), F16)
        xe = stile("xe", (H, N), F16)
        g1 = stile("g1", (HID2, N), F16)
        g2 = stile("g2", (HID2, N), F16)
        od = stile("od", (TOUT, N), F32)

        with tc.tile_pool(name="pG", bufs=3, space="PSUM") as pG, \
             tc.tile_pool(name="pPA", bufs=1, space="PSUM") as pPA, \
             tc.tile_pool(name="pPB", bufs=1, space="PSUM") as pPB, \
             tc.tile_pool(name="ab", bufs=3) as ab, \
             tc.tile_pool(name="pf", bufs=3, space="PSUM") as pf:

            prodq = []       # pending pair products: (pA, pB, Aq, Sq, q, last)
            c2q = []         # deferred C2 stages: (c, pA?, ...) emitted later

            def emit_products():
                if not prodq:
                    return
                pA, pB, Aq, Sq, q, last = prodq.pop(0)
                nc.tensor.matmul(pA[:], xtT[:, q], Aq[:],
                                 start=(q == 0), stop=last, perf_mode=DR)
                nc.tensor.matmul(pB[:], xtT[:, q], Sq[:],
                                 start=(q == 0), stop=last, perf_mode=DR)

            def emit_c2(c):
                # uf/ub broadcast matmuls + xd multiplies (PE + DVE);
                # the recip inputs are long done when these are emitted
                uB = pf.tile([T, 512], F32, name="uB", tag="ps")
                nc.tensor.matmul(uB[:], ones1[:], rrB[:, CS[c]],
                                 start=True, stop=True)
                nc.vector.tensor_tensor(xd[32:44, CS[c]], vf[:, CS[c]],
                                        uB[:], ALU.mult)
                uA = pf.tile([T, 512], F32, name="uA", tag="ps")
                nc.tensor.matmul(uA[:], ones1[:], rrA[:, CS[c]],
                                 start=True, stop=True)
                nc.vector.tensor_tensor(xd[64:76, CS[c]], vb[:, CS[c]],
                                        uA[:], ALU.mult)

            for c in range(NC):
                # ---- B(c): Ghat, compares; pair products lag one pair ----
                pA = pPA.tile([48, 512], F32, name="pA", tag="pA")
                pB = pPB.tile([48, 512], F32, name="pB", tag="pB")
                for q in range(NP):
                    Aq = ab.tile([128, 2, 512], FP8, name="Aq", tag="Aq")
                    Sq = ab.tile([128, 2, 512], FP8, name="Sq", tag="Sq")
                    for m in range(2):
                        isl = slice((2 * q + m) * 128, (2 * q + m + 1) * 128)
                        psG = pG.tile([128, 512], F32, name="psG", tag="g")
                        nc.tensor.matmul(psG[:], phat[:, isl], xs[:, CS[c]],
                                         start=True, stop=True)
                        nc.vector.tensor_scalar(Aq[:, m, :], psG[:], 0.0, None,
                                                ALU.is_gt)
                        psGT = pG.tile([128, 512], F32, name="psGT", tag="g")
                        nc.tensor.matmul(psGT[:], xs[:, isl], phat[:, CS[c]],
                                         start=True, stop=True)
                        nc.scalar.sign(Sq[:, m, :], psGT[:])
                    prodq.append((pA, pB, Aq, Sq, q, q == NP - 1))
                    if q >= 2:
                        emit_products()
                    if q == 0 and c2q:
                        emit_c2(c2q.pop(0))
                emit_products()
                emit_products()

                # ---- C1(c): psum copies + reciprocals (no PE) ----
                nc.scalar.activation(vb[:, CS[c]], pA[0:T, :], AF.Identity)
                nc.scalar.activation(rrA[:, CS[c]], pA[32:33, :], AF.Identity)
                nc.scalar.activation(vf[:, CS[c]], pB[0:T, :], AF.Identity,
                                     bias=bSx[0:T, :])
                nc.scalar.activation(rrB[:, CS[c]], pB[32:33, :],
                                     AF.Identity, bias=bSx[32:33, :])
                from concourse.dve_ops import (RECIP_APPROX_FAST_CONSTS as RC,
                                               RECIPROCAL_APPROX_FAST as RAF)
                nc.vector._custom_dve(RAF, out=rrA[:, CS[c]], in0=rrA[:, CS[c]],
                                      s0=RC["s0"], s1=RC["s1"], imm2=RC["imm2"])
                nc.vector._custom_dve(RAF, out=rrB[:, CS[c]], in0=rrB[:, CS[c]],
                                      s0=RC["s0"], s1=RC["s1"], imm2=RC["imm2"])
                c2q.append(c)

            # ---- D rounds for chunks 0..1, then C2(2), then D(2) ----
            for k in range(6):
                for c in range(NC - 1):
                    ps = pf.tile([128, 512], F32, name="psF", tag="ps")
                    nc.tensor.matmul(ps[:], wb[0:76, O_ES + k * 128:O_ES + (k + 1) * 128],
                                     xd[:, CS[c]], start=True, stop=True)
                    if (k + c) % 2 == 0:
                        nc.scalar.activation(zT[:, k, CS[c]], ps[:], AF.Relu)
                    else:
                        nc.vector.tensor_scalar(zT[:, k, CS[c]], ps[:], 0.0,
                                                None, ALU.max)
                if k == 5 and c2q:
                    emit_c2(c2q.pop(0))
            for k in range(6):
                c = NC - 1
                ps = pf.tile([128, 512], F32, name="psF", tag="ps")
                nc.tensor.matmul(ps[:], wb[0:76, O_ES + k * 128:O_ES + (k + 1) * 128],
                                 xd[:, CS[c]], start=True, stop=True)
                if (k + c) % 2 == 0:
                    nc.scalar.activation(zT[:, k, CS[c]], ps[:], AF.Relu)
                else:
                    nc.vector.tensor_scalar(zT[:, k, CS[c]], ps[:], 0.0,
                                            None, ALU.max)

            # ---- MLP tail rounds ----
            for c in range(NC):
                ps = pf.tile([HID2, 512], F32, name="psH1", tag="ps")
                for k in range(6):
                    nc.tensor.matmul(ps[:], wb[:, O_EW1 + k * 128:O_EW1 + (k + 1) * 128],
                                     zT[:, k, CS[c]], start=(k == 0), stop=(k == 5))
                if c % 2 == 0:
                    nc.scalar.activation(h1[:, CS[c]], ps[:], AF.Relu, bias=bias["eb1"])
                else:
                    nc.vector.tensor_scalar(h1[:, CS[c]], ps[:], bias["eb1"], 0.0,
                                            ALU.add, ALU.max)

            for c in range(NC):
                ps = pf.tile([HID2, 512], F32, name="psH2", tag="ps")
                nc.tensor.matmul(ps[:], wb[:, O_EW2:O_EW2 + HID2], h1[:, CS[c]],
                                 start=True, stop=True)
                if c % 2 == 1:
                    nc.scalar.activation(h2[:, CS[c]], ps[:], AF.Relu, bias=bias["eb2"])
                else:
                    nc.vector.tensor_scalar(h2[:, CS[c]], ps[:], bias["eb2"], 0.0,
                                            ALU.add, ALU.max)

            for c in range(NC):
                ps = pf.tile([H, 512], F32, name="psXe", tag="ps")
                nc.tensor.matmul(ps[:], wb[:, O_EW3:O_EW3 + H], h2[:, CS[c]],
                                 start=True, stop=False)
                for k in range(6):
                    nc.tensor.matmul(ps[:], wb[:, O_EPROJ + k * H:O_EPROJ + (k + 1) * H],
                                     zT[:, k, CS[c]], start=False, stop=(k == 5))
                if c % 2 == 0:
                    nc.scalar.activation(xe[:, CS[c]], ps[:], AF.Identity, bias=bias["ebe"])
                else:
                    nc.vector.tensor_scalar(xe[:, CS[c]], ps[:], bias["ebe"], None,
                                            ALU.add)

            for c in range(NC):
                ps = pf.tile([HID2, 512], F32, name="psG1", tag="ps")
                nc.tensor.matmul(ps[:], wb[0:H, O_DW1:O_DW1 + HID2], xe[:, CS[c]],
                                 start=True, stop=True)
                if c % 2 == 1:
                    nc.scalar.activation(g1[:, CS[c]], ps[:], AF.Relu, bias=bias["db1"])
                else:
                    nc.vector.tensor_scalar(g1[:, CS[c]], ps[:], bias["db1"], 0.0,
                                            ALU.add, ALU.max)

            for c in range(NC):
                ps = pf.tile([HID2, 512], F32, name="psG2", tag="ps")
                nc.tensor.matmul(ps[:], wb[:, O_DW2:O_DW2 + HID2], g1[:, CS[c]],
                                 start=True, stop=True)
                if c % 2 == 0:
                    nc.scalar.activation(g2[:, CS[c]], ps[:], AF.Relu, bias=bias["db2"])
                else:
                    nc.vector.tensor_scalar(g2[:, CS[c]], ps[:], bias["db2"], 0.0,
                                            ALU.add, ALU.max)

            for c in range(NC):
                ps = pf.tile([TOUT, 512], F32, name="psOd", tag="ps")
                nc.tensor.matmul(ps[:], wb[:, O_DW3:O_DW3 + TOUT], g2[:, CS[c]],
                                 start=True, stop=False)
                nc.tensor.matmul(ps[:], wb[0:H, O_DPROJ:O_DPROJ + TOUT], xe[:, CS[c]],
                                 start=False, stop=True)
                if c % 2 == 1:
                    nc.scalar.activation(od[:, CS[c]], ps[:], AF.Identity, bias=bias["dbd"])
                else:
                    nc.vector.tensor_scalar(od[:, CS[c]], ps[:], bias["dbd"], None,
                                            ALU.add)
                eng = (nc.gpsimd, nc.scalar, nc.sync)[c]
                eng.dma_start(out=d["out"].ap()[:, CS[c]], in_=od[:, CS[c]])


def _build_wblob(inputs):
    f32 = np.float32
    f16 = np.float16
    W1 = np.asarray(inputs["W1"], f32)[0]
    W2 = np.asarray(inputs["W2"], f32)[0]
    g = np.asarray(inputs["enc_bn_g"], f32); be = np.asarray(inputs["enc_bn_b"], f32)
    m = np.asarray(inputs["enc_bn_m"], f32); v = np.asarray(inputs["enc_bn_v"], f32)
    esc = g / np.sqrt(v + 1e-5)
    ew3 = np.asarray(inputs["enc_w3"], f32) * esc[None, :]
    eproj = np.asarray(inputs["enc_proj"], f32) * esc[None, :]
    ebe = np.asarray(inputs["enc_b3"], f32) * esc + (be - m * esc)
    g = np.asarray(inputs["dec_bn_g"], f32); bd = np.asarray(inputs["dec_bn_b"], f32)
    m = np.asarray(inputs["dec_bn_m"], f32); v = np.asarray(inputs["dec_bn_v"], f32)
    dsc = g / np.sqrt(v + 1e-5)
    dw3 = np.asarray(inputs["dec_w3"], f32) * dsc[None, :]
    dproj = np.asarray(inputs["dec_proj"], f32) * dsc[None, :]
    dbd = np.asarray(inputs["dec_b3"], f32) * dsc + (bd - m * dsc)

    wb = np.zeros((128, CW), f16)
    # x_flat stack: block-diagonal per t at rows 0/32/64
    for t in range(T):
        wb[t, O_ES + t * H:O_ES + (t + 1) * H] = W1.astype(f16)
        wb[32 + t, O_ES + t * H:O_ES + (t + 1) * H] = (0.9 * W2).astype(f16)
        wb[64 + t, O_ES + t * H:O_ES + (t + 1) * H] = (2.1 * W2).astype(f16)
    ew1 = np.asarray(inputs["enc_w1"], f32)
    for a in range(6):
        wb[:, O_EW1 + a * 128:O_EW1 + (a + 1) * 128] = ew1[a * 128:(a + 1) * 128, :].astype(f16)
        wb[:, O_EPROJ + a * H:O_EPROJ + (a + 1) * H] = eproj[a * 128:(a + 1) * 128, :].astype(f16)
    wb[:, O_EW2:O_EW2 + HID2] = np.asarray(inputs["enc_w2"], f32).astype(f16)
    wb[:, O_EW3:O_EW3 + H] = ew3.astype(f16)
    wb[0:H, O_DW1:O_DW1 + HID2] = np.asarray(inputs["dec_w1"], f32).astype(f16)
    wb[:, O_DW2:O_DW2 + HID2] = np.asarray(inputs["dec_w2"], f32).astype(f16)
    wb[:, O_DW3:O_DW3 + TOUT] = dw3.astype(f16)
    wb[0:H, O_DPROJ:O_DPROJ + TOUT] = dproj.astype(f16)

    def put_f32_col(off, rows, vals):
        col = np.zeros(128, f32)
        col[:rows] = vals
        wb[:, off:off + 2] = col.view(f16).reshape(128, 2)

    put_f32_col(O_EB1, HID2, np.asarray(inputs["enc_b1"], f32))
    put_f32_col(O_EB2, HID2, np.asarray(inputs["enc_b2"], f32))
    put_f32_col(O_EBE, H, ebe)
    put_f32_col(O_DB1, HID2, np.asarray(inputs["dec_b1"], f32))
    put_f32_col(O_DB2, HID2, np.asarray(inputs["dec_b2"], f32))
    put_f32_col(O_DBD, TOUT, dbd)
    return wb


def _weights_fp(inputs):
    """Content fingerprint of every non-x input (cheap; full-content hash)."""
    import hashlib
    h = hashlib.blake2b(digest_size=16)
    for k in sorted(inputs):
        if k == "x":
            continue
        a = np.ascontiguousarray(np.asarray(inputs[k]))
        h.update(k.encode())
        h.update(str(a.shape).encode())
        h.update(a.tobytes())
    return h.digest()


def _make_runner(nc):
    import jax
    from jax.sharding import Mesh, PartitionSpec, NamedSharding
    from jax.experimental.shard_map import shard_map
    from concourse.bass2jax import (_bass_exec_p, install_neuronx_cc_hook,
                                    partition_id_tensor)

    install_neuronx_cc_hook()
    partition_name = nc.partition_id_tensor.name if nc.partition_id_tensor else None

    in_names, out_names, out_avals, zero_shapes = [], [], [], []
    for alloc in nc.m.functions[0].allocations:
        if not isinstance(alloc, mybir.MemoryLocationSet):
            continue
        name = alloc.memorylocations[0].name
        if alloc.kind == "ExternalInput":
            if name != partition_name:
                in_names.append(name)
        elif alloc.kind == "ExternalOutput":
            out_names.append(name)
            shape = tuple(alloc.tensor_shape)
            dtype = mybir.dt.np(alloc.dtype)
            out_avals.append(jax.core.ShapedArray(shape, dtype))
            zero_shapes.append((shape, dtype))
    n_params = len(in_names)
    all_in_names = tuple(in_names + out_names + ([partition_name] if partition_name else []))

    def _body(*args):
        operands = list(args)
        if partition_name is not None:
            operands.append(partition_id_tensor())
        outs = _bass_exec_p.bind(
            *operands,
            out_avals=tuple(out_avals),
            in_names=all_in_names,
            out_names=tuple(out_names),
            lowering_input_output_aliases=(),
            sim_require_finite=True,
            sim_require_nnan=True,
            nc=nc,
        )
        return tuple(outs)

    devices = jax.devices()[:B]
    mesh = Mesh(np.asarray(devices), ("core",))
    nin = n_params + len(out_names)
    sharded = jax.jit(
        shard_map(_body, mesh=mesh, in_specs=(PartitionSpec("core"),) * nin,
                  out_specs=(PartitionSpec("core"),) * len(out_names), check_rep=False),
        keep_unused=True,
    )
    sh = NamedSharding(mesh, PartitionSpec("core"))
    zeros = [jax.device_put(np.zeros((B * s[0], *s[1:]), dt), sh)
             for (s, dt) in zero_shapes]
    return sharded, zeros


def _build_ctx(inputs):
    wb = _build_wblob(inputs)
    nc = _build_nc(wb)
    sharded, zeros = _make_runner(nc)
    return {"fp": _weights_fp(inputs), "nc": nc, "sharded": sharded, "zeros": zeros,
            "ids": tuple(id(inputs[k]) for k in sorted(inputs) if k != "x")}


def kernel(**inputs) -> np.ndarray:
    ctx = _cache.get("ctx")
    if ctx is not None:
        ids = tuple(id(inputs[k]) for k in sorted(inputs) if k != "x")
        if ids != ctx["ids"]:
            if _weights_fp(inputs) == ctx["fp"]:
                ctx["ids"] = ids
            else:
                ctx = None
    if ctx is None:
        ctx = _build_ctx(inputs)
        _cache["ctx"] = ctx

    x = np.asarray(inputs["x"], np.float32).reshape(B * T, N)
    out = ctx["sharded"](x, *ctx["zeros"])[0]
    return np.asarray(out).reshape(B, TOUT, N, 1).astype(np.float32, copy=False)
